# revision 43
# baseline (speedup 1.0000x reference)
"""Trainium2 Bass kernel for nn_Attn (Luong 'general'-score attention softmax).

reference:
    energy[b,l,:] = targets[b,l,:] @ W.T + bias          # [B, L, H]
    s[b,l]        = energy[b,l,:] . h[b,:]               # [B, L]
    out           = softmax(s, axis=1)[:, None, :]       # [B, 1, L]

Algebraic refactor (exact up to fp rounding):
    s[b,l] = targets[b,l,:] . v[b,:] + (h[b,:].bias)
    with v[b,:] = h[b,:] @ W.  The per-row constant h.bias cancels in
    softmax, so it is dropped entirely.  This turns a B*L*H*H matmul into
    a B*L*H batched row-dot + tiny H*H matvec: the kernel is then purely
    HBM-bandwidth-bound on streaming `targets` once.

Layout (per core, data-parallel over batch, 4 batches/core):
    v      = h_loc @ W on PE (W streamed in 4 chunks, matmuls pipelined)
    vrep   = v[b] replicated over 128 partitions via one-hot PE matmul +
             ACT copy (PSUM -> SBUF)
    s      : for each l-tile [128l, 1024h]: ONE fused DVE
             scalar_tensor_tensor (mult + free-dim add accumulator)
             against vrep -> S_all[:, col]; stream is DMA-paced at
             ~410 GB/s with 8-deep chunk prefetch
    softmax: per-b max via free-reduce + PE transpose; exp + row-sums in a
             single ACT activation(Exp, bias=-max, accum_out); per-b sums
             and broadcasts via tiny selector matmuls on PE; final scale on
             DVE; one contiguous DMA out.

Env quirks handled here: this walrus build lowers at most one sem-wait
per instruction (extra waits are hoisted to EventSemaphore instructions
by a BIR rewrite before compile), and raw-ISA DVE ops (e.g.
tensor_tensor_reduce) are rejected, hence the native
scalar_tensor_tensor.
"""

import json

import numpy as np

import concourse.bass as bass
import concourse.tile as tile
from concourse import bass2jax, bass_utils, mybir
from concourse.bass_utils import run_bass_kernel_spmd

F32 = mybir.dt.float32
B, L, H = 32, 4096, 1024
NCORES = 8
BPC = B // NCORES          # batches per core
NT = L // 128              # 128-row l-tiles per batch (32)
CPACK_F = 292 + 4 * 128    # packed-consts free size (ident|hT|selp|selb|bsel)
LCHUNK = 512               # l rows per targets DMA (2 MiB)
NJ = LCHUNK // 128         # sub-tiles per chunk
NCH = L // LCHUNK          # chunks per batch
TGT_BUFS = 8               # deep prefetch so DMA never stalls on DVE


def _split_multiwaits(bir_json):
    """The walrus build here lowers at most ONE sem-wait per instruction;
    hoist extra waits into standalone EventSemaphore instructions inserted
    just before the owner (same engine => same in-order stream)."""
    bir = json.loads(bir_json)
    for fn in bir["functions"]:
        for blk in fn["blocks"]:
            new_insts = []
            for ins in blk.get("instructions", []):
                si = ins.get("sync_info")
                ow = (si or {}).get("on_wait") or []
                if len(ow) > 1:
                    for k, w in enumerate(ow[:-1]):
                        new_insts.append(
                            {
                                "debug": ins.get("debug", 0),
                                "engine": ins["engine"],
                                "ins": [],
                                "name": f"{ins['name']}_hw{k}",
                                "opcode": "EventSemaphore",
                                "outs": [],
                                "sync_info": {"on_update": [], "on_wait": [w]},
                            }
                        )
                    si["on_wait"] = [ow[-1]]
                new_insts.append(ins)
            blk["instructions"] = new_insts
    return json.dumps(bir).encode()


_ORIG_COMPILE_BIR = bass_utils.compile_bir_kernel


def _compile_bir_split(bir_json, tmpdir, neff_name="file.neff"):
    return _ORIG_COMPILE_BIR(_split_multiwaits(bir_json), tmpdir, neff_name=neff_name)


def _patch_compile():
    bass_utils.compile_bir_kernel = _compile_bir_split
    bass2jax.compile_bir_kernel = _compile_bir_split


def _patch_tile_drain():
    """walrus in this env only lowers 1 sem-wait per TPB_CTRL Drain; split
    the TileContext exit-drain waits into individual wait_ge instructions."""
    if getattr(tile.TileContext, "_drain_patched", False):
        return

    def _drain_and_barrier(self, tick_clock, wait_clock):
        nc = self.nc
        drain_inst = nc.sync.drain()
        wait_clock.add_sem_waits(
            drain_inst.ins, tile.ScopedClock({None: tick_clock.global_clock})
        )
        si = drain_inst.ins.sync_info
        waits = list(si.on_wait or [])
        if len(waits) > 1:
            si.on_wait = []
            handles = {}
            for h in self.sems.allocated().values():
                handles[getattr(h, "name", None) or str(h)] = h
            for ww in waits:
                nc.sync.wait_ge(handles[ww.ant_name], ww.wait_value)
        nc.all_engine_barrier()
        popped = nc._tile_sem_poison_stack.pop()
        assert popped is self._sem_poison
        nc.clear_and_free_semaphores(list(self.sems.allocated().values()))
        nc.all_engine_barrier()

    tile.TileContext._drain_and_barrier = _drain_and_barrier
    tile.TileContext._drain_patched = True


def build_kernel(tc, tgt, W, cpack, out):
    nc = tc.nc
    mult = mybir.AluOpType.mult
    amax = mybir.AluOpType.max
    AX = mybir.AxisListType.X

    import contextlib

    ctx = contextlib.ExitStack()
    consts = ctx.enter_context(tc.tile_pool(name="consts", bufs=1))
    tgtp = ctx.enter_context(tc.tile_pool(name="tgtp", bufs=TGT_BUFS))
    prodp = ctx.enter_context(tc.tile_pool(name="prodp", bufs=1))
    smallp = ctx.enter_context(tc.tile_pool(name="smallp", bufs=1))
    psump = ctx.enter_context(tc.tile_pool(name="psump", bufs=4, space="PSUM"))

    _psctr = [0]

    def pstile(shape):
        _psctr[0] += 1
        return psump.tile(shape, F32, tag="ps", name=f"ps{_psctr[0]}")

    # ---- all small constants in ONE packed DMA (see make_in_maps) ----
    cpack_sb = consts.tile([128, CPACK_F], F32)
    nc.sync.dma_start(out=cpack_sb, in_=cpack)
    ident_sb = cpack_sb[:, 0:128]
    hT_sb = cpack_sb[:, 128:160].rearrange("p (c b) -> p c b", b=BPC)
    selp_sb = cpack_sb[:, 160 : 160 + BPC]
    selb_sb = cpack_sb[0:BPC, 164:292]
    bsel_sb = cpack_sb[0:BPC, 292 : 292 + BPC * 128].rearrange(
        "p (b m) -> p b m", m=128
    )

    # Preload the exp table set during the DMA phase so the epilogue
    # doesn't pay the ~2.7us ACT_TABLE_LOAD on the critical path.
    warm = smallp.tile([1, 1], F32)
    nc.scalar.activation(
        out=warm, in_=ident_sb[0:1, 0:1], func=mybir.ActivationFunctionType.Exp
    )

    # ---- v = h_loc @ W -> [BPC, H]; W DMA'd in 4 chunks pipelined with
    # the per-chunk accumulation matmuls ----
    W_sb = consts.tile([128, 8, H], F32)
    v_ps0 = pstile([BPC, 512])
    v_ps1 = pstile([BPC, 512])
    vps = [v_ps0, v_ps1]
    for cc in range(4):
        nc.sync.dma_start(
            out=W_sb[:, 2 * cc : 2 * cc + 2, :],
            in_=W[cc * 256 : (cc + 1) * 256, :].rearrange(
                "(c p) h -> p c h", p=128
            ),
        )
        for c in (2 * cc, 2 * cc + 1):
            for n in range(H // 512):
                nc.tensor.matmul(
                    vps[n],
                    lhsT=hT_sb[:, c, :],
                    rhs=W_sb[:, c, n * 512 : (n + 1) * 512],
                    start=(c == 0),
                    stop=(c == 7),
                )
    v_sb = smallp.tile([BPC, H], F32)
    nc.vector.tensor_copy(v_sb[:, 0:512], vps[0])
    nc.scalar.copy(out=v_sb[:, 512:1024], in_=vps[1])

    # vrep[b] = v[b] replicated across all 128 partitions, via one-hot
    # PE matmul (bsel[:, b, :].T @ v_sb) + ACT copy back to SBUF.
    vrep = consts.tile([128, BPC, H], F32)
    for b in range(BPC):
        for nh in range(H // 512):
            vb_ps = pstile([128, 512])
            nc.tensor.matmul(
                vb_ps,
                lhsT=bsel_sb[:, b, :],
                rhs=v_sb[:, nh * 512 : (nh + 1) * 512],
                start=True,
                stop=True,
            )
            nc.scalar.copy(
                out=vrep[:, b, nh * 512 : (nh + 1) * 512], in_=vb_ps
            )

    # ---- main loop: s[b, l] = targets[b, l, :] . v[b] ----
    # S_all[p, b*NT + t] = s[b, t*128 + p]
    S_all = smallp.tile([128, BPC * NT], F32)
    for b in range(BPC):
        for ch in range(NCH):
            tg = tgtp.tile([128, NJ, H], F32)
            nc.sync.dma_start(
                out=tg,
                in_=tgt[b, ch * LCHUNK : (ch + 1) * LCHUNK, :].rearrange(
                    "(j p) h -> p j h", p=128
                ),
            )
            for j in range(NJ):
                col = b * NT + ch * NJ + j
                pr = prodp.tile([128, H], F32)
                nc.vector.scalar_tensor_tensor(
                    out=pr,
                    in0=tg[:, j, :],
                    scalar=1.0,
                    in1=vrep[:, b, :],
                    op0=mult,
                    op1=mult,
                    accum_out=S_all[:, col : col + 1],
                )

    # ---- softmax over l (4096) per batch ----
    # per-(p, b) max over the NT tiles (split per b so they run mid-stream)
    pm = smallp.tile([128, BPC], F32)
    for b in range(BPC):
        nc.vector.tensor_reduce(
            pm[:, b : b + 1],
            S_all[:, b * NT : (b + 1) * NT],
            axis=AX,
            op=amax,
        )
    # cross-partition max: transpose then free-reduce
    pmt = pstile([BPC, 128])
    nc.tensor.transpose(pmt, pm, ident_sb)
    negm4 = smallp.tile([BPC, 1], F32)
    nc.vector.tensor_reduce(negm4, pmt, axis=AX, op=amax, negate=True)
    # broadcast -max[b] to the [128] chunk-partition layout: selb.T @ negm4
    negmb_ps = pstile([128, 1])
    nc.tensor.matmul(negmb_ps, lhsT=selb_sb, rhs=negm4, start=True, stop=True)
    negmb = smallp.tile([128, 1], F32)
    nc.vector.tensor_copy(negmb, negmb_ps)

    # transpose scores to chunk-partition layout: S_t[b*NT+t, p] = s[b, t*128+p]
    st_ps = psump.tile([128, 128], F32, tag="st", name="st_ps", bufs=1)
    nc.tensor.transpose(st_ps, S_all, ident_sb)
    # E = exp(s - max_b); R[p'] = sum_f E[p', f]
    E = smallp.tile([128, 128], F32)
    R = smallp.tile([128, 1], F32)
    nc.scalar.activation(
        out=E,
        in_=st_ps,
        func=mybir.ActivationFunctionType.Exp,
        bias=negmb,
        scale=1.0,
        accum_out=R,
    )
    # per-b denominator: selp.T @ R -> [BPC, 1]
    s4_ps = pstile([BPC, 1])
    nc.tensor.matmul(s4_ps, lhsT=selp_sb, rhs=R, start=True, stop=True)
    r4 = smallp.tile([BPC, 1], F32)
    nc.vector.reciprocal(r4, s4_ps)
    # broadcast 1/denom back to [128] chunk-partitions
    rb_ps = pstile([128, 1])
    nc.tensor.matmul(rb_ps, lhsT=selb_sb, rhs=r4, start=True, stop=True)
    rb = smallp.tile([128, 1], F32)
    nc.vector.tensor_copy(rb, rb_ps)

    O = smallp.tile([128, 128], F32)
    nc.vector.tensor_scalar_mul(O, E, rb)
    # out[b, t*128 + f] = O[b*NT + t, f]; flat layout is contiguous
    nc.sync.dma_start(out=out.rearrange("b (t f) -> (b t) f", f=128), in_=O)
    ctx.close()


def build_bass():
    _patch_tile_drain()
    _patch_compile()
    nc = bass.Bass("TRN2", target_bir_lowering=False, debug=False, num_devices=NCORES)
    tgt = nc.dram_tensor("tgt", [BPC, L, H], F32, kind="ExternalInput").ap()
    W_t = nc.dram_tensor("W", [H, H], F32, kind="ExternalInput").ap()
    cpack = nc.dram_tensor("cpack", [128, CPACK_F], F32, kind="ExternalInput").ap()
    out = nc.dram_tensor("out", [BPC, L], F32, kind="ExternalOutput").ap()
    with tile.TileContext(nc) as tc:
        build_kernel(tc, tgt, W_t, cpack, out)
    return nc


def make_in_maps(hidden, targets, W):
    h = np.ascontiguousarray(hidden[0], dtype=np.float32)          # [B, H]
    W = np.ascontiguousarray(W, dtype=np.float32)
    ident = np.eye(128, dtype=np.float32)
    selb = np.zeros((BPC, 128), np.float32)
    for b in range(BPC):
        selb[b, b * NT : (b + 1) * NT] = 1.0
    selp = selb.T.copy()
    bsel = np.zeros((BPC, BPC, 128), np.float32)
    for b in range(BPC):
        bsel[b, b, :] = 1.0
    in_maps = []
    for c in range(NCORES):
        bl = slice(c * BPC, (c + 1) * BPC)
        # packed consts blob: [128, CPACK_F]
        # cols 0:128 ident | 128:160 hT as (p, c, b) | 160:164 selp |
        # 164:292 selb (rows 0..3) | 292:292+512 bsel (rows 0..3)
        cp = np.zeros((128, CPACK_F), np.float32)
        cp[:, 0:128] = ident
        hTl = h[bl].T.reshape(8, 128, BPC).transpose(1, 0, 2)  # [p, c, b]
        cp[:, 128:160] = hTl.reshape(128, 8 * BPC)
        cp[:, 160 : 160 + BPC] = selp
        cp[0:BPC, 164:292] = selb
        cp[0:BPC, 292 : 292 + BPC * 128] = bsel.reshape(BPC, BPC * 128)
        in_maps.append(
            {
                "tgt": np.ascontiguousarray(targets[bl], dtype=np.float32),
                "W": W,
                "cpack": cp,
            }
        )
    return in_maps


_CACHED_NC = None


def kernel(hidden, targets, W, b, _trace=False):
    global _CACHED_NC
    if _CACHED_NC is None:
        _CACHED_NC = build_bass()
    nc = _CACHED_NC
    in_maps = make_in_maps(hidden, targets, W)
    res = run_bass_kernel_spmd(nc, in_maps, list(range(NCORES)), trace=_trace)
    out = np.concatenate([res.results[c]["out"] for c in range(NCORES)], axis=0)
    kernel.last_results = res
    return out.reshape(B, 1, L).astype(np.float32)
